# revision 1
# baseline (speedup 1.0000x reference)
"""Trainium2 Bass kernel for pairwise radial-angular graph convolution.

Computes, for z in 0..3 (batch), a,b in 0..511 (points), i,j in 0..15:
  rel = g[z,b] - g[z,a];  d = sqrt(|rel|^2 + eps)
  rad_r = exp(-gamma*(d - c_r)^2)          (8 radial shells)
  ang   = [1, rel/d]                        (4 angular fns)
  out[z,a,i] = 1/sqrt(n) * sum_{b,r,m,j} rad_r*ang_m*W[r,m,i,j]*f[z,b,j]

Restructured to avoid the [z,a,b,i,j] HBM intermediate entirely:
  G[b, r, (S|Vb|V)] = [f; g_c*f]^T @ wext   (PE, K=64, zero-padded blocks)
  -g*d^2[b,a] = 6-row factored matmul       (PE, K=6)
  d, 1/d      = exp(+-0.5*ln(d^2+eps))      (ACT, one combined table set)
  rad'_r      = exp(2*g*c_r*d - g*d^2)      (DVE arg + ACT exp; exp(-g c_r^2)
                                             folded into W on host)
  q'_r        = rad'_r / d                  (DVE/GPSIMD, diag-masked)
  psum1[i,a]  += S_r^T rad'_r               (PE, K=b, float32r streams)
  psum2[vi,a] += [Vb|V]_r^T q'_r            (PE, K=b)
  out[i,a] = psum1 + psum2[Vb] - sum_c gA_c[a]*psum2[V_c]  (DVE + selector MM)

Sharding: 8 cores = 4 z x 2 a-halves; inputs replicated per-z; full output
gathered on host.
"""

import math

import numpy as np

# ---------------------------------------------------------------- constants
Z, NPTS, C_IN, C_OUT = 4, 512, 16, 16
NUM_RADIAL, NUM_ANGULAR = 8, 4
MAX_R, GAMMA = 3.0, 8.0
EPS = 1e-12
N_CORES = 8
A_PER_CORE = NPTS // 2          # 256 output points per core
N_BT = NPTS // 128              # 4 b-tiles of 128
CENTERS = [MAX_R * r / (NUM_RADIAL - 1) for r in range(NUM_RADIAL)]

# knobs (radial shells 0..N_STT-1 take the DVE stt path; the rest use ACT
# Square; q-mults are split DVE [0, N_QDVE) / GPSIMD [N_QDVE, 8))
N_STT = 6
N_STT_GPS = 0            # stt on GPSIMD unsupported (walrus rejects
                         # TensorScalarPtr on Pool) — keep 0
N_QDVE = 3
MM_F32R = True           # contraction matmuls in float32r (1 cyc/row at
                         # N>=256 vs 4 for fp32)

_CACHE = {}


def _build_program():
    import concourse.bacc as bacc
    import concourse.mybir as mybir
    import concourse.tile as tile

    f32 = mybir.dt.float32
    f32s = mybir.dt.float32r if MM_F32R else f32   # stream/stationary dtype
    AF = mybir.ActivationFunctionType
    ALU = mybir.AluOpType

    nc = bacc.Bacc("TRN2", target_bir_lowering=False, debug=False)

    # register activation-bias constants (same pattern as Bass.__init__)
    bias_vals = {EPS} | {-CENTERS[r] for r in range(N_STT, NUM_RADIAL)}
    for v in sorted(bias_vals):
        t = nc.alloc_sbuf_tensor(f"const-f32-{v}", [128, 1], f32)
        nc.gpsimd.memset(t.ap(), v)
        nc.const_aps.aps[(f32, v)] = t.ap()
    nc.all_engine_barrier()

    # ---------------- IO -------------------------------------------------
    b6_d = nc.dram_tensor("b6", [6, NPTS], f32, kind="ExternalInput")
    a6_d = nc.dram_tensor("a6", [6, A_PER_CORE], f32, kind="ExternalInput")
    fext_d = nc.dram_tensor("fext", [64, NPTS], f32, kind="ExternalInput")
    wext_d = nc.dram_tensor("wext", [64, NUM_RADIAL * 80], f32,
                            kind="ExternalInput")
    gasel_d = nc.dram_tensor("gasel", [6, 64], f32, kind="ExternalInput")
    sel64_d = nc.dram_tensor("sel64", [64, 16], f32, kind="ExternalInput")
    dmask_d = nc.dram_tensor("dmask", [128, N_BT * A_PER_CORE], f32,
                             kind="ExternalInput")
    out_d = nc.dram_tensor("out", [16, A_PER_CORE], f32, kind="ExternalOutput")

    A = A_PER_CORE
    with tile.TileContext(nc) as tc:
        with (
            tc.tile_pool(name="const", bufs=1) as cpool,
            tc.tile_pool(name="gsb", bufs=N_BT) as gpool,
            tc.tile_pool(name="work", bufs=3) as wpool,
            tc.tile_pool(name="big", bufs=3) as bigpool,
            tc.tile_pool(name="fin", bufs=1) as fpool,
            tc.tile_pool(name="mmps", bufs=3, space="PSUM") as mmps,
            tc.tile_pool(name="acc1", bufs=1, space="PSUM") as acc1p,
            tc.tile_pool(name="acc2", bufs=1, space="PSUM") as acc2p,
        ):
            # ---------------- load inputs -------------------------------
            b6 = cpool.tile([6, NPTS], f32, tag="b6")
            a6 = cpool.tile([6, A], f32, tag="a6")
            fext = cpool.tile([64, NPTS], f32, tag="fext")
            wext = cpool.tile([64, NUM_RADIAL * 80], f32, tag="wext")
            gasel = cpool.tile([6, 64], f32, tag="gasel")
            sel64 = cpool.tile([64, 16], f32, tag="sel64")
            dmask = cpool.tile([128, N_BT * A], f32, tag="dmask")
            for t, d in ((b6, b6_d), (a6, a6_d), (fext, fext_d),
                         (wext, wext_d), (gasel, gasel_d), (sel64, sel64_d),
                         (dmask, dmask_d)):
                nc.sync.dma_start(out=t[:], in_=d.ap())

            # pre-load the combined ln+exp+square activation table set so
            # the insert_act_table_loads pass doesn't alternate between the
            # ln-only and exp-only sets (one ~2.7us load instead of five)
            preload = mybir.InstLoadActFuncSet(
                name=nc.get_next_instruction_name(),
                act_func_set_id=6, ins=[], outs=[])
            preload.engine = mybir.EngineType.Activation
            nc.scalar.add_instruction(preload)

            # ------- gA broadcast [64, A]: rows 0:16 = 1, 16+16c+i = -gA_c
            ga_ps = mmps.tile([64, A], f32, tag="mm")
            nc.tensor.matmul(ga_ps[:], gasel[:], a6[:], start=True, stop=True)
            ga_sb = fpool.tile([64, A], f32, tag="gasb")
            nc.vector.tensor_copy(ga_sb[:], ga_ps[:])

            # -------- G per b-tile: [128, r*80 + (S16 | Vb16 | V48)] -----
            g_sb = []
            for t in range(N_BT):
                bsl = slice(t * 128, (t + 1) * 128)
                gsb = gpool.tile([128, NUM_RADIAL * 80], f32s, tag="g")
                psA = mmps.tile([128, 320], f32, tag="mm")
                nc.tensor.matmul(psA[:], fext[:, bsl], wext[:, 0:320],
                                 start=True, stop=True)
                nc.vector.tensor_copy(gsb[:, 0:320], psA[:])
                psB = mmps.tile([128, 320], f32, tag="mm")
                nc.tensor.matmul(psB[:], fext[:, bsl], wext[:, 320:640],
                                 start=True, stop=True)
                nc.vector.tensor_copy(gsb[:, 320:640], psB[:])
                g_sb.append(gsb)

            # ---------------- accumulators ------------------------------
            psum1 = acc1p.tile([16, A], f32, tag="p1")   # rad * S
            psum2 = acc2p.tile([64, A], f32, tag="p2")   # q * [Vb | V]

            # ---------------- main loop over b-tiles --------------------
            for t in range(N_BT):
                bsl = slice(t * 128, (t + 1) * 128)
                # -gamma*d^2 via factored K=6 matmul
                ndps = mmps.tile([128, A], f32, tag="mm")
                nc.tensor.matmul(ndps[:], b6[:, bsl], a6[:],
                                 start=True, stop=True)
                nd2 = wpool.tile([128, A], f32, tag="nd2")
                # clamp tiny positive fp error so ln arg stays >= 0
                nc.vector.tensor_scalar_min(nd2[:], ndps[:], 0.0)

                lt = wpool.tile([128, A], f32, tag="lt")
                nc.scalar.activation(lt[:], nd2[:], AF.Ln,
                                     bias=EPS, scale=-1.0 / GAMMA)
                dd = wpool.tile([128, A], f32, tag="dd")
                nc.scalar.activation(dd[:], lt[:], AF.Exp, scale=0.5)
                rcp = wpool.tile([128, A], f32, tag="rcp")
                nc.scalar.activation(rcp[:], lt[:], AF.Exp, scale=-0.5)
                # zero 1/d on the a==b diagonal: angular term is exactly 0
                # there, but fp32 Vb-vs-V cancellation otherwise blows up
                rcpm = wpool.tile([128, 1, A], f32, tag="rcpm")
                nc.gpsimd.tensor_mul(rcpm[:, 0, :], rcp[:],
                                     dmask[:, t * A:(t + 1) * A])

                # exp arguments, one [128, A] slice per radial shell
                arg = bigpool.tile([128, NUM_RADIAL * A], f32, tag="arg")
                for r in range(NUM_RADIAL):
                    asl = slice(r * A, (r + 1) * A)
                    if r >= N_STT:
                        # (d - c_r)^2 ; exp scale -gamma applied below
                        nc.scalar.activation(arg[:, asl], dd[:], AF.Square,
                                             bias=-CENTERS[r])
                    else:
                        # 2*gamma*c_r*d + (-gamma*d^2)
                        eng = nc.gpsimd if r < N_STT_GPS else nc.vector
                        eng.scalar_tensor_tensor(
                            arg[:, asl], dd[:], 2.0 * GAMMA * CENTERS[r],
                            nd2[:], ALU.mult, ALU.add)

                rad = bigpool.tile([128, NUM_RADIAL * A], f32s, tag="rad")
                nc.scalar.activation(rad[:, 0:N_STT * A],
                                     arg[:, 0:N_STT * A], AF.Exp)
                nc.scalar.activation(rad[:, N_STT * A:],
                                     arg[:, N_STT * A:], AF.Exp, scale=-GAMMA)

                # q_r = rad_r / d, batched with broadcast 1/d
                q = bigpool.tile([128, NUM_RADIAL * A], f32s, tag="q")
                ndve = N_QDVE
                nc.vector.tensor_mul(
                    q[:, 0:ndve * A].rearrange("p (r a) -> p r a", a=A),
                    rad[:, 0:ndve * A].rearrange("p (r a) -> p r a", a=A),
                    rcpm[:].to_broadcast([128, ndve, A]))
                nc.gpsimd.tensor_mul(
                    q[:, ndve * A:].rearrange("p (r a) -> p r a", a=A),
                    rad[:, ndve * A:].rearrange("p (r a) -> p r a", a=A),
                    rcpm[:].to_broadcast([128, NUM_RADIAL - ndve, A]))

                # contraction matmuls
                for r in range(NUM_RADIAL):
                    asl = slice(r * A, (r + 1) * A)
                    first = (t == 0 and r == 0)
                    last = (t == N_BT - 1 and r == NUM_RADIAL - 1)
                    # term1: rad * S_r -> psum1 [16, A]
                    nc.tensor.matmul(psum1[:],
                                     g_sb[t][:, r * 80:r * 80 + 16],
                                     rad[:, asl],
                                     start=first, stop=False)
                    # terms 2+3: q * [Vb_r | V_rc] -> psum2 [64, A]
                    nc.tensor.matmul(psum2[:],
                                     g_sb[t][:, r * 80 + 16:(r + 1) * 80],
                                     q[:, asl],
                                     start=first, stop=last)

            # ---------------- final combine -----------------------------
            p2sb = fpool.tile([64, A], f32, tag="p2sb")
            nc.vector.tensor_copy(p2sb[:], psum2[:])
            w1 = fpool.tile([64, A], f32, tag="w1")
            nc.vector.tensor_mul(w1[:], ga_sb[:], p2sb[:])
            # psum1 += w1[Vb] + sum_c -gA_c*w1[V_c] via selector matmul
            nc.tensor.matmul(psum1[:], sel64[:], w1[:], start=False, stop=True)
            osb = fpool.tile([16, A], f32, tag="osb")
            nc.vector.tensor_copy(osb[:], psum1[:])
            nc.sync.dma_start(out=out_d.ap(), in_=osb[:])

    nc.compile()
    return nc


def _host_prep(features, geometry, W, n_norm):
    """Build per-core input maps (all small host-side tensors)."""
    f = np.asarray(features, dtype=np.float32)
    g = np.asarray(geometry, dtype=np.float32)
    W = np.asarray(W, dtype=np.float32)
    scale = 1.0 / math.sqrt(float(n_norm))

    # fold 1/sqrt(n) and, for stt-path shells, exp(-gamma c_r^2) into W
    Wp = W.astype(np.float64) * scale
    for r in range(N_STT):
        Wp[r] *= math.exp(-GAMMA * CENTERS[r] ** 2)
    Wp = Wp.astype(np.float32)

    # wext [64, r*80 + (S16 | Vb16 | V48)]:
    #   S  cols: rows 0:16 (j)        = Wp[r, 0, i, j]
    #   Vb cols: rows 16+16c+j        = Wp[r, c+1, i, j]
    #   V  cols: rows 0:16 (j)        = Wp[r, c+1, i, j] at col 16c+i
    wext = np.zeros((64, NUM_RADIAL * 80), dtype=np.float32)
    for r in range(NUM_RADIAL):
        base = r * 80
        wext[0:16, base:base + 16] = Wp[r, 0].T                  # [j, i]
        for c in range(3):
            wext[16 + 16 * c:32 + 16 * c, base + 16:base + 32] = Wp[r, c + 1].T
            wext[0:16, base + 32 + 16 * c:base + 48 + 16 * c] = Wp[r, c + 1].T

    gasel = np.zeros((6, 64), dtype=np.float32)
    gasel[5, 0:16] = 1.0
    for c in range(3):
        gasel[c, 16 + 16 * c:32 + 16 * c] = -1.0 / (2.0 * GAMMA)
    sel64 = np.zeros((64, 16), dtype=np.float32)
    for blk in range(4):
        sel64[16 * blk:16 * (blk + 1)] = np.eye(16, dtype=np.float32)

    in_maps = []
    for core in range(N_CORES):
        z, half = core // 2, core % 2
        gz = g[z]                                    # [512, 3]
        fz = f[z]                                    # [512, 16]
        a0 = half * A_PER_CORE
        ga = gz[a0:a0 + A_PER_CORE]                  # [256, 3]

        b6 = np.empty((6, NPTS), dtype=np.float32)
        b6[0:3] = gz.T
        b6[3] = (gz * gz).sum(axis=1)
        b6[4] = 1.0
        b6[5] = 0.0

        a6 = np.empty((6, A_PER_CORE), dtype=np.float32)
        a6[0:3] = 2.0 * GAMMA * ga.T
        a6[3] = -GAMMA
        a6[4] = -GAMMA * (ga * ga).sum(axis=1)
        a6[5] = 1.0

        fext = np.empty((64, NPTS), dtype=np.float32)
        fext[0:16] = fz.T
        for c in range(3):
            fext[16 + 16 * c:32 + 16 * c] = (fz * gz[:, c:c + 1]).T

        dmask = np.ones((128, N_BT * A_PER_CORE), dtype=np.float32)
        for t in range(N_BT):
            lo = max(a0, t * 128) - a0
            for p in range(128):
                col = t * 128 + p - a0
                if 0 <= col < A_PER_CORE:
                    dmask[p, t * A_PER_CORE + col] = 0.0

        in_maps.append({
            "b6": b6, "a6": a6, "fext": np.ascontiguousarray(fext),
            "wext": wext, "gasel": gasel, "sel64": sel64, "dmask": dmask,
        })
    return in_maps


def kernel(features, geometry, W, n_norm):
    from concourse.bass_utils import run_bass_kernel_spmd

    if "nc" not in _CACHE:
        _CACHE["nc"] = _build_program()
    nc = _CACHE["nc"]

    in_maps = _host_prep(features, geometry, W, n_norm)
    res = run_bass_kernel_spmd(nc, in_maps, list(range(N_CORES)))

    out = np.empty((Z, NPTS, C_OUT), dtype=np.float32)
    for core in range(N_CORES):
        z, half = core // 2, core % 2
        o = res.results[core]["out"]                 # [16, 256]
        out[z, half * A_PER_CORE:(half + 1) * A_PER_CORE, :] = o.T
    return out

